# revision 9
# baseline (speedup 1.0000x reference)
"""Trainium2 Bass kernel for DeepSeek-V3-style block-sparse MoE MLP.

Strategy (expert-parallel, 8 cores; dense expert 0 + capacity-sparse 1-3):
  - Each core owns 4 of the 32 experts (fp16 weights). Local expert 0 is
    computed DENSE over all 256 tokens (it depends only on x + weights, so
    the PE starts the moment wg0 lands, hiding the routing latency).
    Experts 1-3 are computed SPARSE: tokens are gathered into 128 capacity
    slots per expert with one-hot matmuls (max actual count is 96), the MLP
    runs on the gathered [128, H] activations, and results are scattered
    back with routing weights folded into the scatter matrix
    (Gw[c,t] = rw[t,e] * (rank_e(t)==c)).
  - Routing is replicated on every core (hi/lo fp16 split-precision logits:
    min 8th-vs-9th expert margin is 1.06e-4, below plain-fp16 logit error,
    so the split is mandatory). Ranks come from triangular-ones prefix-sum
    matmuls; the gather one-hot G^T is built from a strided DVE reduce +
    Pool-engine iota compares so the critical chain avoids transposes.
    The SPMD program is identical on every core (only selbc/lsel/lselm
    inputs differ per core).
  - All matmuls fp16 (fp32 accumulate). Host sums the 8 partial outputs.
"""
import sys
sys.path.insert(0, '/opt/trn_rl_repo')
import numpy as np
import concourse.mybir as mybir
import concourse.tile as tile
from concourse import bass
from concourse.bass_utils import run_bass_kernel_spmd

T, H, I, E = 256, 1024, 512, 32
N_CORES = 8
E_LOC = E // N_CORES            # 4 experts per core
N_SP = E_LOC - 1                # sparse experts per core (locals 1..3)
N_GROUP, GSZ = 8, 4             # 8 groups of 4 experts
ROUTED_SCALING_FACTOR = 2.5
P = 128
C = 128                         # capacity slots per expert (max count is 96)
SLOTS = N_SP * C                # 384 gather slots per core
NTT = T // P                    # token tiles
NHC = H // P                    # h chunks (contraction for up/gate proj)
NIC = I // P                    # i chunks (contraction for down proj)
HH = H // 512                   # h halves for down-proj PSUM banks
dt = mybir.dt
F32, BF = dt.float32, dt.float16
Alu = mybir.AluOpType
Act = mybir.ActivationFunctionType

_CACHE = {}


def _build():
    nc = bass.Bass('TRN2')
    xtb_d = nc.dram_tensor('xtb', [P, NHC * T], BF, kind='ExternalInput')
    xtlo_d = nc.dram_tensor('xtlo', [P, NHC * T], BF, kind='ExternalInput')
    gcat_d = nc.dram_tensor('gcat', [P, NHC * 2 * E], BF, kind='ExternalInput')
    biasb_d = nc.dram_tensor('biasb', [P, E], F32, kind='ExternalInput')
    xnat_d = nc.dram_tensor('xnat', [P, NTT * H], BF, kind='ExternalInput')
    selbc_d = nc.dram_tensor('selbc', [E, E_LOC * P], BF, kind='ExternalInput')
    lselm_d = nc.dram_tensor('lselm', [P, E], F32, kind='ExternalInput')
    # wg/wu: [p, e, c, i]; wd: [p, e, ic, h]
    wg_d = nc.dram_tensor('wg', [P, E_LOC * NHC * I], BF, kind='ExternalInput')
    wu_d = nc.dram_tensor('wu', [P, E_LOC * NHC * I], BF, kind='ExternalInput')
    wd_d = nc.dram_tensor('wd', [P, E_LOC * NIC * H], BF, kind='ExternalInput')
    out_d = nc.dram_tensor('out', [T, H], BF, kind='ExternalOutput')

    WSEG = NHC * I
    DSEG = NIC * H

    with tile.TileContext(nc) as tc:
        with tc.tile_pool(name='consts', bufs=1) as consts, \
             tc.tile_pool(name='wpool', bufs=1) as wpool, \
             tc.tile_pool(name='rt', bufs=2) as rt, \
             tc.tile_pool(name='actp', bufs=2) as actp, \
             tc.tile_pool(name='atp', bufs=1) as atp, \
             tc.tile_pool(name='ygp', bufs=1) as ygp, \
             tc.tile_pool(name='outp', bufs=1) as outp, \
             tc.tile_pool(name='ps', bufs=1, space='PSUM') as ps, \
             tc.tile_pool(name='psy', bufs=1, space='PSUM') as psy:

            def pst(nm):
                # single rotating PSUM ring: 4 x [128, 512] fp32 banks
                return ps.tile([P, 512], F32, name=nm, tag='ps', bufs=4)

            # ---------- PE warmup --------------------------------------
            scratch_bf = consts.tile([P, 512], BF)
            nc.vector.memset(scratch_bf, 0.0)
            pwarm = pst('pwarm')
            for i in range(2):
                nc.tensor.matmul(pwarm, lhsT=scratch_bf[:, 0:128],
                                 rhs=scratch_bf, start=(i == 0), stop=(i == 1))

            # ---------- iota constants (Pool engine) -------------------
            iota_col = consts.tile([P, 1], F32)       # partition index
            nc.gpsimd.iota(iota_col, pattern=[[0, 1]], channel_multiplier=1,
                           allow_small_or_imprecise_dtypes=True)
            iota_row = consts.tile([P, C], F32)       # free index 0..127
            nc.gpsimd.iota(iota_row, pattern=[[1, C]], channel_multiplier=0,
                           allow_small_or_imprecise_dtypes=True)
            iota_row1 = consts.tile([P, C], F32)      # free index 1..128
            nc.gpsimd.iota(iota_row1, pattern=[[1, C]], base=1,
                           channel_multiplier=0,
                           allow_small_or_imprecise_dtypes=True)

            # ---------- input DMAs -------------------------------------
            xtb_sb = consts.tile([P, NHC, T], BF)
            xtlo_sb = consts.tile([P, NHC, T], BF)
            gcat_sb = consts.tile([P, NHC, 2 * E], BF)
            biasb_sb = consts.tile([P, E], F32)
            xnat_sb = consts.tile([P, NTT, H], BF)
            selbc_sb = consts.tile([E, E_LOC * P], BF)
            lselm_sb = consts.tile([P, E], F32)
            wg_sb, wu_sb, wd_sb = [], [], []
            for e in range(E_LOC):
                wg_sb.append(wpool.tile([P, NHC, I], BF, name=f'wg{e}', tag=f'wg{e}'))
                wu_sb.append(wpool.tile([P, NHC, I], BF, name=f'wu{e}', tag=f'wu{e}'))
                wd_sb.append(wpool.tile([P, NIC, H], BF, name=f'wd{e}', tag=f'wd{e}'))

            def dma_gu(w_sb, w_d, e):
                nc.sync.dma_start(
                    w_sb[e].rearrange("p c i -> p (c i)"),
                    w_d[:, e * WSEG:(e + 1) * WSEG])

            def dma_wd(e, hh=None):
                if hh is None:
                    nc.sync.dma_start(
                        wd_sb[e].rearrange("p c h -> p (c h)"),
                        wd_d[:, e * DSEG:(e + 1) * DSEG])
                else:
                    # one h-half of wd[e]: [P, NIC, 512] strided in dram
                    nc.sync.dma_start(
                        wd_sb[e][:, :, hh * 512:(hh + 1) * 512],
                        wd_d.rearrange("p (e c h) -> p e c h", e=E_LOC, c=NIC)
                        [:, e, :, hh * 512:(hh + 1) * 512])

            # main ring (need-order); tiny tensors go on the Pool DGE ring
            nc.sync.dma_start(gcat_sb.rearrange("p c e -> p (c e)"), gcat_d[:, :])
            nc.sync.dma_start(xtb_sb.rearrange("p c t -> p (c t)"), xtb_d[:, :])
            nc.gpsimd.dma_start(biasb_sb, biasb_d[:, :])
            nc.gpsimd.dma_start(selbc_sb, selbc_d[:, :])
            nc.gpsimd.dma_start(lselm_sb, lselm_d[:, :])
            dma_gu(wg_sb, wg_d, 0)
            dma_gu(wu_sb, wu_d, 0)
            nc.sync.dma_start(xtlo_sb.rearrange("p c t -> p (c t)"), xtlo_d[:, :])
            nc.sync.dma_start(xnat_sb.rearrange("p t h -> p (t h)"), xnat_d[:, :])
            dma_gu(wg_sb, wg_d, 1)
            dma_gu(wu_sb, wu_d, 1)
            dma_wd(0)
            dma_gu(wg_sb, wg_d, 2)
            dma_gu(wu_sb, wu_d, 2)
            dma_wd(1)
            dma_gu(wg_sb, wg_d, 3)
            dma_gu(wu_sb, wu_d, 3)
            dma_wd(2)
            dma_wd(3, 0)
            dma_wd(3, 1)

            # out PSUM tiles (also double as router-logit scratch: the pl
            # groups finish before down_dense opens fresh groups there)
            yps = [psy.tile([P, 512], F32, name=f'y{tt}_{hh}', tag=f'y{tt}_{hh}')
                   for tt in range(NTT) for hh in range(HH)]
            pls = [yps[0], yps[1]]

            # ---------- router logits: hi both tiles ASAP --------------
            for tt in range(NTT):
                tsl = slice(tt * P, (tt + 1) * P)
                for c in range(NHC):
                    nc.tensor.matmul(pls[tt][:, 0:2 * E], lhsT=xtb_sb[:, c, tsl],
                                     rhs=gcat_sb[:, c, :],
                                     start=(c == 0), stop=False)

            # ---------- dense expert 0 gate (hides routing latency) ----
            pgu0 = []
            for ic in range(NIC):
                pgu = pst(f'pgu0_{ic}')
                pgu0.append(pgu)
                for c in range(NHC):
                    nc.tensor.matmul(pgu[:, 0:T], lhsT=wg_sb[0][:, c, ic * P:(ic + 1) * P],
                                     rhs=xtb_sb[:, c, :],
                                     start=(c == 0), stop=(c == NHC - 1))

            # ---------- router logits: lo correction -------------------
            for tt in range(NTT):
                tsl = slice(tt * P, (tt + 1) * P)
                for c in range(NHC):
                    nc.tensor.matmul(pls[tt][:, 0:E], lhsT=xtlo_sb[:, c, tsl],
                                     rhs=gcat_sb[:, c, 0:E],
                                     start=False, stop=(c == NHC - 1))

            # ---------- routing DVE chain (per token tile) -------------
            rwT_sb = consts.tile([E, T], F32)
            selm16_sb = consts.tile([P, NTT, E], BF)
            selm_f32 = []
            for tt in range(NTT):
                pl = pls[tt]
                lhalf = rt.tile([P, E], F32, name='lhalf', tag='lhalf')
                nc.vector.tensor_copy(lhalf, pl[:, E:2 * E])
                lsum = rt.tile([P, E], F32, name='lsum', tag='lsum')
                nc.vector.tensor_add(lsum, pl[:, 0:E], lhalf)
                scores = rt.tile([P, E], F32, name='scores', tag='scores')
                nc.scalar.activation(scores, lsum, Act.Sigmoid)
                s4c = rt.tile([P, E], F32, name='s4c', tag='s4c')
                nc.vector.tensor_add(s4c, scores, biasb_sb)

                # group score: sum of top-2 of each group of 4
                s4c3 = s4c.rearrange("p (g j) -> p g j", j=GSZ)
                v = [s4c3[:, :, j] for j in range(GSZ)]
                m1 = rt.tile([P, N_GROUP], F32, name='m1', tag='m1')
                n1 = rt.tile([P, N_GROUP], F32, name='n1', tag='n1')
                m2 = rt.tile([P, N_GROUP], F32, name='m2', tag='m2')
                n2 = rt.tile([P, N_GROUP], F32, name='n2', tag='n2')
                nc.vector.tensor_tensor(m1, v[0], v[1], op=Alu.max)
                nc.vector.tensor_tensor(n1, v[0], v[1], op=Alu.min)
                nc.vector.tensor_tensor(m2, v[2], v[3], op=Alu.max)
                nc.vector.tensor_tensor(n2, v[2], v[3], op=Alu.min)
                top1 = rt.tile([P, N_GROUP], F32, name='top1', tag='top1')
                mn = rt.tile([P, N_GROUP], F32, name='mn', tag='mn')
                mx2 = rt.tile([P, N_GROUP], F32, name='mx2', tag='mx2')
                sec = rt.tile([P, N_GROUP], F32, name='sec', tag='sec')
                nc.vector.tensor_tensor(top1, m1, m2, op=Alu.max)
                nc.vector.tensor_tensor(mn, m1, m2, op=Alu.min)
                nc.vector.tensor_tensor(mx2, n1, n2, op=Alu.max)
                nc.vector.tensor_tensor(sec, mn, mx2, op=Alu.max)
                gsc = rt.tile([P, N_GROUP], F32, name='gsc', tag='gsc')
                nc.vector.tensor_add(gsc, top1, sec)

                g8 = rt.tile([P, 8], F32, name='g8', tag='g8')
                nc.vector.max(g8, gsc)
                gmask = rt.tile([P, N_GROUP], F32, name='gmask', tag='gmask')
                nc.vector.tensor_scalar(gmask, gsc, g8[:, 3:4], None, op0=Alu.is_ge)

                masked = rt.tile([P, E], F32, name='masked', tag='masked')
                masked3 = masked.rearrange("p (g j) -> p g j", j=GSZ)
                for j in range(GSZ):
                    nc.vector.tensor_tensor(masked3[:, :, j], v[j], gmask,
                                            op=Alu.mult)
                t8 = rt.tile([P, 8], F32, name='t8', tag='t8')
                nc.vector.max(t8, masked)
                selm = rt.tile([P, E], F32, name='selm', tag='selm')
                nc.vector.tensor_scalar(selm, masked, t8[:, 7:8], None,
                                        op0=Alu.is_ge)
                selm_f32.append(selm)
                nc.vector.tensor_copy(selm16_sb[:, tt, :], selm)

                rw_raw = rt.tile([P, E], F32, name='rw_raw', tag='rw_raw')
                nc.vector.tensor_tensor(rw_raw, scores, selm, op=Alu.mult)
                den = rt.tile([P, 1], F32, name='den', tag='den')
                nc.vector.tensor_reduce(den, rw_raw, axis=mybir.AxisListType.X,
                                        op=Alu.add)
                inv = rt.tile([P, 1], F32, name='inv', tag='inv')
                nc.vector.reciprocal(inv, den)
                rw = rt.tile([P, E], F32, name='rw', tag='rw')
                nc.vector.tensor_scalar(rw, rw_raw, inv,
                                        ROUTED_SCALING_FACTOR,
                                        op0=Alu.mult, op1=Alu.mult)
                for i in range(4):
                    nc.vector.transpose(
                        rwT_sb[:, tt * P + 32 * i:tt * P + 32 * (i + 1)],
                        rw[32 * i:32 * (i + 1), :])
            rwT16 = consts.tile([E, T], BF)
            nc.vector.tensor_copy(rwT16, rwT_sb)

            # ---------- dense expert 0: up proj + silu (Act engine) ----
            sg0_sb = consts.tile([P, NIC, T], F32)
            pu0_sb = consts.tile([P, NIC, T], F32)
            for ic in range(NIC):
                pgu = pgu0[ic]
                for c in range(NHC):
                    nc.tensor.matmul(pgu[:, T:2 * T], lhsT=wu_sb[0][:, c, ic * P:(ic + 1) * P],
                                     rhs=xtb_sb[:, c, :],
                                     start=(c == 0), stop=(c == NHC - 1))
                nc.scalar.activation(sg0_sb[:, ic, :], pgu[:, 0:T], Act.Silu)
                nc.scalar.copy(pu0_sb[:, ic, :], pgu[:, T:2 * T])

            # ---------- ranks: exclusive prefix-sum over tokens --------
            lstrict = consts.tile([P, P], BF)
            nc.vector.tensor_scalar(lstrict, iota_row, iota_col, None,
                                    op0=Alu.is_gt)
            ones128 = consts.tile([P, P], BF)
            nc.vector.memset(ones128, 1.0)
            r2 = pst('r2')
            nc.tensor.matmul(r2[:, 0:E], lhsT=lstrict,
                             rhs=selm16_sb[:, 0, :], start=True, stop=True)
            nc.tensor.matmul(r2[:, E:2 * E], lhsT=ones128,
                             rhs=selm16_sb[:, 0, :], start=True, stop=False)
            nc.tensor.matmul(r2[:, E:2 * E], lhsT=lstrict,
                             rhs=selm16_sb[:, 1, :], start=False, stop=True)

            # local ranks (rank+1 encoding): u2 = (R2+1)*selm*lselm, then
            # per-local-expert sum over the 8 group columns (one-hot mask)
            rloc1_sb = consts.tile([P, NTT, E_LOC], F32)
            for tt in range(NTT):
                selmM = rt.tile([P, E], F32, name='selmM', tag='selmM')
                nc.vector.tensor_mul(selmM, selm_f32[tt], lselm_sb)
                u2 = rt.tile([P, E], F32, name='u2', tag='u2')
                nc.vector.scalar_tensor_tensor(
                    u2, r2[:, tt * E:(tt + 1) * E], 1.0, selmM,
                    op0=Alu.add, op1=Alu.mult)
                u2v = u2.rearrange("p (k j) -> p j k", j=E_LOC)
                for j in range(1, E_LOC):
                    nc.vector.tensor_reduce(rloc1_sb[:, tt, j:j + 1],
                                            u2v[:, j, :],
                                            axis=mybir.AxisListType.X,
                                            op=Alu.add)

            # R2' = (R2+1)*selm - 1 (global, for the scatter-side Gw)
            r2p_sb = consts.tile([P, NTT, E], F32)
            for tt in range(NTT):
                u = rt.tile([P, E], F32, name='u', tag='u')
                nc.vector.scalar_tensor_tensor(
                    u, r2[:, tt * E:(tt + 1) * E], 1.0, selm_f32[tt],
                    op0=Alu.add, op1=Alu.mult)
                nc.vector.tensor_scalar(r2p_sb[:, tt, :], u, -1.0, None,
                                        op0=Alu.add)

            # gather one-hot G^T[t, slot] (Pool engine, off the DVE chain)
            gT_sb = consts.tile([P, NTT, SLOTS], BF)
            for tt in range(NTT):
                for e in range(1, E_LOC):
                    nc.gpsimd.tensor_scalar(
                        gT_sb[:, tt, (e - 1) * C:e * C], iota_row1,
                        rloc1_sb[:, tt, e:e + 1], None, op0=Alu.is_equal)

            # ---------- token gather: xgT[h, slot] ---------------------
            xgT_sb = consts.tile([P, NHC, SLOTS], BF)
            for hc in range(NHC):
                g = pst(f'g{hc}')
                for tt in range(NTT):
                    nc.tensor.matmul(g[:, 0:SLOTS],
                                     lhsT=xnat_sb[:, tt, hc * P:(hc + 1) * P],
                                     rhs=gT_sb[:, tt, :],
                                     start=(tt == 0), stop=(tt == NTT - 1))
                if hc % 2 == 0:
                    nc.vector.tensor_copy(xgT_sb[:, hc, :], g[:, 0:SLOTS])
                else:
                    nc.scalar.copy(xgT_sb[:, hc, :], g[:, 0:SLOTS])

            # ---------- dense expert 0: rw fold (Pool engine) ----------
            rwb0p = pst('rwb0')
            nc.tensor.matmul(rwb0p[:, 0:T], lhsT=selbc_sb[:, 0:P],
                             rhs=rwT16, start=True, stop=True)
            rwb0_sb = consts.tile([P, T], F32)
            nc.scalar.copy(rwb0_sb, rwb0p[:, 0:T])
            at0 = atp.tile([P, NIC, T], BF, name='at0', tag='at0')
            for ic in range(NIC):
                t1d = rt.tile([P, T], F32, name=f't1d{ic}', tag='t1d')
                nc.gpsimd.tensor_mul(t1d, sg0_sb[:, ic, :], pu0_sb[:, ic, :])
                nc.gpsimd.tensor_mul(at0[:, ic, :], t1d, rwb0_sb)

            # ---------- transposed ranks (scatter-side, off-chain) -----
            r2T_sb = consts.tile([E, T], F32)
            for tt in range(NTT):
                for i in range(4):
                    nc.vector.transpose(
                        r2T_sb[:, tt * P + 32 * i:tt * P + 32 * (i + 1)],
                        r2p_sb[32 * i:32 * (i + 1), tt, :])
            r2T16 = consts.tile([E, T], BF)
            nc.vector.tensor_copy(r2T16, r2T_sb)

            # ---------- sparse experts ---------------------------------
            at_sb = {}
            ygsb = {}
            pgs = {}

            def emit_gu(e):
                pg = pst(f'pg{e}')
                pu = pst(f'pu{e}')
                pgs[e] = (pg, pu)
                xg = xgT_sb[:, :, (e - 1) * C:e * C]
                for it in range(NIC):
                    for hc in range(NHC):
                        nc.tensor.matmul(pg[:, it * C:(it + 1) * C],
                                         lhsT=wg_sb[e][:, hc, it * P:(it + 1) * P],
                                         rhs=xg[:, hc, :],
                                         start=(hc == 0), stop=(hc == NHC - 1))
                for it in range(NIC):
                    for hc in range(NHC):
                        nc.tensor.matmul(pu[:, it * C:(it + 1) * C],
                                         lhsT=wu_sb[e][:, hc, it * P:(it + 1) * P],
                                         rhs=xg[:, hc, :],
                                         start=(hc == 0), stop=(hc == NHC - 1))

            def emit_at(e):
                pg, pu = pgs[e]
                sg = actp.tile([P, NIC * C], F32, name=f'sg{e}', tag='sg')
                nc.scalar.activation(sg, pg, Act.Silu)
                at = atp.tile([P, NIC, C], BF, name=f'at{e}', tag='at', bufs=2)
                nc.vector.tensor_mul(at.rearrange("p i c -> p (i c)"), sg, pu)
                at_sb[e] = at

            gw_sb = consts.tile([P, N_SP, T], BF)

            def emit_gw(e):
                rbc = pst(f'rbc{e}')
                nc.tensor.matmul(rbc[:, 0:T],
                                 lhsT=selbc_sb[:, e * P:(e + 1) * P],
                                 rhs=r2T16, start=True, stop=True)
                nc.tensor.matmul(rbc[:, T:2 * T],
                                 lhsT=selbc_sb[:, e * P:(e + 1) * P],
                                 rhs=rwT16, start=True, stop=True)
                eq = rt.tile([P, T], F32, name=f'eq{e}', tag='eq')
                nc.vector.tensor_scalar(eq, rbc[:, 0:T], iota_col, None,
                                        op0=Alu.is_equal)
                nc.vector.tensor_tensor(gw_sb[:, e - 1, :], eq, rbc[:, T:2 * T],
                                        op=Alu.mult)

            def emit_down_dense():
                # expert 0 dense: writes [t, h] directly into out PSUM
                for tt in range(NTT):
                    for hh in range(HH):
                        for it in range(NIC):
                            nc.tensor.matmul(
                                yps[tt * HH + hh],
                                lhsT=at0[:, it, tt * P:(tt + 1) * P],
                                rhs=wd_sb[0][:, it, hh * 512:(hh + 1) * 512],
                                start=(it == 0), stop=False)

            def emit_down(e, hh_list=None):
                if e not in ygsb:
                    ygsb[e] = ygp.tile([P, HH, 512], BF, name=f'ygsb{e}',
                                       tag='ygsb', bufs=2)
                yg = ygsb[e]
                for hh in (hh_list if hh_list is not None else range(HH)):
                    p = pst(f'yg{e}_{hh}')
                    for it in range(NIC):
                        nc.tensor.matmul(p, lhsT=at_sb[e][:, it, :],
                                         rhs=wd_sb[e][:, it, hh * 512:(hh + 1) * 512],
                                         start=(it == 0), stop=(it == NIC - 1))
                    if hh == 0:
                        nc.vector.tensor_copy(yg[:, hh, :], p)
                    else:
                        nc.scalar.copy(yg[:, hh, :], p)

            osbs = [outp.tile([P, 512], BF, name=f'osb{tt}_{hh}',
                              tag=f'osb{tt}_{hh}')
                    for tt in range(NTT) for hh in range(HH)]

            def drain_out(tt, hh):
                k = tt * HH + hh
                if (tt + hh) % 2 == 0:
                    nc.vector.tensor_copy(osbs[k], yps[k])
                else:
                    nc.scalar.copy(osbs[k], yps[k])
                nc.sync.dma_start(
                    out_d[tt * P:(tt + 1) * P, hh * 512:(hh + 1) * 512],
                    osbs[k])

            def emit_scatter(e, hh_list=None, drain=False):
                last = (e == E_LOC - 1)
                for hh in (hh_list if hh_list is not None else range(HH)):
                    for tt in range(NTT):
                        nc.tensor.matmul(
                            yps[tt * HH + hh],
                            lhsT=gw_sb[:, e - 1, tt * P:(tt + 1) * P],
                            rhs=ygsb[e][:, hh, :],
                            start=False, stop=last)
                    if drain:
                        for tt in range(NTT):
                            drain_out(tt, hh)

            # PE order: dense expert 0 first (gu emitted above), sparse
            # experts pipelined behind their weight DMAs; wd3 split by
            # h-half so the tail after the last DMA byte is minimal.
            emit_gu(1)
            emit_down_dense()
            for e in range(1, E_LOC):
                emit_gw(e)
            emit_at(1)
            emit_gu(2)
            emit_at(2)
            emit_down(1)
            emit_scatter(1)
            emit_gu(3)
            emit_at(3)
            emit_down(2)
            emit_scatter(2)
            emit_down(3, [0])
            emit_scatter(3, [0], drain=True)
            emit_down(3, [1])
            emit_scatter(3, [1], drain=True)

    _spill_excess_waits(nc)
    return nc


def _spill_excess_waits(nc, max_waits=1):
    """walrus codegen in this container accepts at most one semaphore wait
    per engine instruction; move extra waits onto preceding same-engine NOPs
    (engine queues are in-order, so this preserves the synchronization)."""
    f = nc.m.functions[0]
    for b in f.blocks:
        new_insts = []
        for inst in b.instructions:
            si = inst.sync_info
            if si is not None and si.on_wait is not None \
                    and len(si.on_wait) > max_waits:
                waits = list(si.on_wait)
                keep = waits[-max_waits:]
                extra = waits[:-max_waits]
                for k, w in enumerate(extra):
                    nop = mybir.InstNoOp(
                        name=f"{inst.name}-wspill{k}",
                        sync_info=mybir.SyncInfo(on_wait=[w], on_update=[]),
                        bass_nofuse=True,
                        engine=inst.engine,
                    )
                    new_insts.append(nop)
                inst.sync_info = mybir.SyncInfo(
                    on_wait=keep, on_update=list(si.on_update or []))
            new_insts.append(inst)
        b.instructions = new_insts


def kernel(x, gate_w, e_score_bias, Wg, Wu, Wd):
    if 'nc' not in _CACHE:
        _CACHE['nc'] = _build()
    nc = _CACHE['nc']

    f16 = np.float16

    def pmajor_ht(a):
        n = a.shape[1]
        return np.ascontiguousarray(
            a.reshape(NHC, P, n).transpose(1, 0, 2).reshape(P, NHC * n))

    xf = np.asarray(x).astype(np.float32)
    xT = np.ascontiguousarray(xf.T)
    xTb = xT.astype(f16)
    xTlo = (xT - xTb.astype(np.float32)).astype(f16)
    xnat = np.ascontiguousarray(
        xf.astype(f16).reshape(NTT, P, H).transpose(1, 0, 2).reshape(P, -1))
    gate = np.ascontiguousarray(np.asarray(gate_w)).astype(np.float32)
    ghi = gate.astype(f16)
    glo = (gate - ghi.astype(np.float32)).astype(f16)
    gcat = np.concatenate([ghi, glo], axis=1)          # [H, 2E]
    biasb = np.broadcast_to(
        np.asarray(e_score_bias).astype(np.float32)[None, :], (P, E)).copy()
    Wgb = np.asarray(Wg).astype(f16).reshape(E, NHC, P, I)
    Wgb = np.ascontiguousarray(Wgb.transpose(2, 0, 1, 3))      # [P,E,NHC,I]
    Wub = np.asarray(Wu).astype(f16).reshape(E, NHC, P, I)
    Wub = np.ascontiguousarray(Wub.transpose(2, 0, 1, 3))
    Wdb = np.asarray(Wd).astype(f16).reshape(E, NIC, P, H)
    Wdb = np.ascontiguousarray(Wdb.transpose(2, 0, 1, 3))      # [P,E,NIC,H]

    in_maps = []
    for c in range(N_CORES):
        sel = np.zeros((E, E_LOC, P), dtype=f16)
        lselm = np.zeros((E,), dtype=np.float32)
        for j in range(E_LOC):
            sel[c * E_LOC + j, j, :] = 1.0
            lselm[c * E_LOC + j] = 1.0
        esl = slice(c * E_LOC, (c + 1) * E_LOC)
        in_maps.append({
            'xtb': pmajor_ht(xTb),
            'xtlo': pmajor_ht(xTlo),
            'gcat': pmajor_ht(gcat),
            'biasb': biasb,
            'xnat': xnat,
            'selbc': sel.reshape(E, E_LOC * P),
            'lselm': np.broadcast_to(lselm[None, :], (P, E)).copy(),
            'wg': np.ascontiguousarray(Wgb[:, esl]).reshape(P, -1),
            'wu': np.ascontiguousarray(Wub[:, esl]).reshape(P, -1),
            'wd': np.ascontiguousarray(Wdb[:, esl]).reshape(P, -1),
        })

    _CACHE['in_maps'] = in_maps
    res = run_bass_kernel_spmd(nc, in_maps, core_ids=list(range(N_CORES)))
    out = np.zeros((T, H), dtype=np.float32)
    for c in range(N_CORES):
        out += res.results[c]['out'].astype(np.float32)
    return out


def run_traced(**kwargs):
    """Re-run the last kernel invocation with NTFF tracing enabled."""
    return run_bass_kernel_spmd(_CACHE['nc'], _CACHE['in_maps'],
                                core_ids=list(range(N_CORES)), trace=True,
                                **kwargs)


# revision 16
# speedup vs baseline: 1.1413x; 1.1413x over previous
"""Trainium2 Bass kernel for DeepSeek-V3-style block-sparse MoE MLP.

Strategy (expert-parallel, 8 cores; dense expert 0 + capacity-sparse 1-3):
  - Each core owns 4 of the 32 experts (fp16 weights). Local expert 0 is
    computed DENSE over all 256 tokens (it depends only on x + weights, so
    the PE starts the moment wg0 lands, hiding the routing latency).
    Experts 1-3 are computed SPARSE: tokens are gathered into 128 capacity
    slots per expert with one-hot matmuls (max actual count is 96), the MLP
    runs on the gathered [128, H] activations, and results are scattered
    back with routing weights folded into the scatter matrix
    (Gw[c,t] = rw[t,e] * (rank_e(t)==c)).
  - Routing is replicated on every core (hi/lo fp16 split-precision logits:
    min 8th-vs-9th expert margin is 1.06e-4, below plain-fp16 logit error,
    so the split is mandatory). Ranks come from triangular-ones prefix-sum
    matmuls; the gather one-hot G^T is built from a strided DVE reduce +
    Pool-engine iota compares so the critical chain avoids transposes.
    The SPMD program is identical on every core (only selbc/lsel/lselm
    inputs differ per core).
  - All matmuls fp16 (fp32 accumulate). Host sums the 8 partial outputs.
"""
import sys
sys.path.insert(0, '/opt/trn_rl_repo')
import numpy as np
import concourse.mybir as mybir
import concourse.tile as tile
from concourse import bass
from concourse.bass_utils import run_bass_kernel_spmd

T, H, I, E = 256, 1024, 512, 32
N_CORES = 8
E_LOC = E // N_CORES            # 4 experts per core
N_SP = E_LOC - 1                # sparse experts per core (locals 1..3)
N_GROUP, GSZ = 8, 4             # 8 groups of 4 experts
ROUTED_SCALING_FACTOR = 2.5
P = 128
C = 128                         # capacity slots per expert (max count is 96)
SLOTS = N_SP * C                # 384 gather slots per core
NTT = T // P                    # token tiles
NHC = H // P                    # h chunks (contraction for up/gate proj)
NIC = I // P                    # i chunks (contraction for down proj)
HH = H // 512                   # h halves for down-proj PSUM banks
dt = mybir.dt
F32, BF = dt.float32, dt.float16
Alu = mybir.AluOpType
Act = mybir.ActivationFunctionType

_CACHE = {}


def _build():
    nc = bass.Bass('TRN2')
    xtb_d = nc.dram_tensor('xtb', [P, NHC * T], BF, kind='ExternalInput')
    xtlo_d = nc.dram_tensor('xtlo', [P, NHC * T], BF, kind='ExternalInput')
    gcat_d = nc.dram_tensor('gcat', [P, NHC * 2 * E], BF, kind='ExternalInput')
    biasb_d = nc.dram_tensor('biasb', [P, E], F32, kind='ExternalInput')
    xnat_d = nc.dram_tensor('xnat', [P, NTT * H], BF, kind='ExternalInput')
    selbc_d = nc.dram_tensor('selbc', [E, E_LOC * P], BF, kind='ExternalInput')
    lselm_d = nc.dram_tensor('lselm', [P, E], F32, kind='ExternalInput')
    # wg/wu: [p, e, c, i]; wd: [p, e, ic, h]
    wg_d = nc.dram_tensor('wg', [P, E_LOC * NHC * I], BF, kind='ExternalInput')
    wu_d = nc.dram_tensor('wu', [P, E_LOC * NHC * I], BF, kind='ExternalInput')
    wd_d = nc.dram_tensor('wd', [P, E_LOC * NIC * H], BF, kind='ExternalInput')
    out_d = nc.dram_tensor('out', [T, H], BF, kind='ExternalOutput')

    WSEG = NHC * I
    DSEG = NIC * H

    with tile.TileContext(nc) as tc:
        with tc.tile_pool(name='consts', bufs=1) as consts, \
             tc.tile_pool(name='wpool', bufs=1) as wpool, \
             tc.tile_pool(name='rt', bufs=2) as rt, \
             tc.tile_pool(name='actp', bufs=2) as actp, \
             tc.tile_pool(name='atp', bufs=1) as atp, \
             tc.tile_pool(name='ygp', bufs=1) as ygp, \
             tc.tile_pool(name='outp', bufs=1) as outp, \
             tc.tile_pool(name='ps', bufs=1, space='PSUM') as ps, \
             tc.tile_pool(name='psy', bufs=1, space='PSUM') as psy:

            def pst(nm):
                # single rotating PSUM ring: 4 x [128, 512] fp32 banks
                return ps.tile([P, 512], F32, name=nm, tag='ps', bufs=4)

            # ---------- PE warmup --------------------------------------
            scratch_bf = consts.tile([P, 512], BF)
            nc.vector.memset(scratch_bf, 0.0)
            pwarm = pst('pwarm')
            for i in range(2):
                nc.tensor.matmul(pwarm, lhsT=scratch_bf[:, 0:128],
                                 rhs=scratch_bf, start=(i == 0), stop=(i == 1))

            # ---------- iota constants (Pool engine) -------------------
            iota_col = consts.tile([P, 1], F32)       # partition index
            nc.gpsimd.iota(iota_col, pattern=[[0, 1]], channel_multiplier=1,
                           allow_small_or_imprecise_dtypes=True)
            iota_row = consts.tile([P, C], F32)       # free index 0..127
            nc.gpsimd.iota(iota_row, pattern=[[1, C]], channel_multiplier=0,
                           allow_small_or_imprecise_dtypes=True)
            iota_row1 = consts.tile([P, C], F32)      # free index 1..128
            nc.gpsimd.iota(iota_row1, pattern=[[1, C]], base=1,
                           channel_multiplier=0,
                           allow_small_or_imprecise_dtypes=True)

            # ---------- input DMAs -------------------------------------
            xtb_sb = consts.tile([P, NHC, T], BF)
            xtlo_sb = consts.tile([P, NHC, T], BF)
            gcat_sb = consts.tile([P, NHC, 2 * E], BF)
            biasb_sb = consts.tile([P, E], F32)
            xnat_sb = consts.tile([P, NTT, H], BF)
            selbc_sb = consts.tile([E, E_LOC * P], BF)
            lselm_sb = consts.tile([P, E], F32)
            wg_sb, wu_sb, wd_sb = [], [], []
            for e in range(E_LOC):
                wg_sb.append(wpool.tile([P, NHC, I], BF, name=f'wg{e}', tag=f'wg{e}'))
                wu_sb.append(wpool.tile([P, NHC, I], BF, name=f'wu{e}', tag=f'wu{e}'))
                wd_sb.append(wpool.tile([P, NIC, H], BF, name=f'wd{e}', tag=f'wd{e}'))

            def dma_gu(w_sb, w_d, e):
                nc.sync.dma_start(
                    w_sb[e].rearrange("p c i -> p (c i)"),
                    w_d[:, e * WSEG:(e + 1) * WSEG])

            def dma_wd(e, hh=None):
                if hh is None:
                    nc.sync.dma_start(
                        wd_sb[e].rearrange("p c h -> p (c h)"),
                        wd_d[:, e * DSEG:(e + 1) * DSEG])
                else:
                    # one h-half of wd[e]: [P, NIC, 512] strided in dram
                    nc.sync.dma_start(
                        wd_sb[e][:, :, hh * 512:(hh + 1) * 512],
                        wd_d.rearrange("p (e c h) -> p e c h", e=E_LOC, c=NIC)
                        [:, e, :, hh * 512:(hh + 1) * 512])

            # main ring (need-order); tiny tensors go on the Pool DGE ring
            nc.sync.dma_start(gcat_sb.rearrange("p c e -> p (c e)"), gcat_d[:, :])
            nc.sync.dma_start(xtb_sb.rearrange("p c t -> p (c t)"), xtb_d[:, :])
            nc.gpsimd.dma_start(biasb_sb, biasb_d[:, :])
            nc.gpsimd.dma_start(selbc_sb, selbc_d[:, :])
            nc.gpsimd.dma_start(lselm_sb, lselm_d[:, :])
            dma_gu(wg_sb, wg_d, 0)
            dma_gu(wu_sb, wu_d, 0)
            nc.sync.dma_start(xtlo_sb.rearrange("p c t -> p (c t)"), xtlo_d[:, :])
            nc.sync.dma_start(xnat_sb.rearrange("p t h -> p (t h)"), xnat_d[:, :])
            dma_gu(wg_sb, wg_d, 1)
            dma_gu(wu_sb, wu_d, 1)
            dma_wd(0)
            dma_gu(wg_sb, wg_d, 2)
            dma_gu(wu_sb, wu_d, 2)
            dma_wd(1)
            dma_gu(wg_sb, wg_d, 3)
            dma_gu(wu_sb, wu_d, 3)
            dma_wd(2)
            dma_wd(3, 0)
            dma_wd(3, 1)

            # out PSUM tiles (also double as router-logit scratch: the pl
            # groups finish before down_dense opens fresh groups there)
            yps = [psy.tile([P, 512], F32, name=f'y{tt}_{hh}', tag=f'y{tt}_{hh}')
                   for tt in range(NTT) for hh in range(HH)]
            pls = [yps[0], yps[1]]

            # ---------- router logits: hi both tiles ASAP --------------
            for tt in range(NTT):
                tsl = slice(tt * P, (tt + 1) * P)
                for c in range(NHC):
                    nc.tensor.matmul(pls[tt][:, 0:2 * E], lhsT=xtb_sb[:, c, tsl],
                                     rhs=gcat_sb[:, c, :],
                                     start=(c == 0), stop=False)

            # ---------- dense expert 0 gate (hides routing latency) ----
            pgu0 = []
            for ic in range(NIC):
                pgu = pst(f'pgu0_{ic}')
                pgu0.append(pgu)
                for c in range(NHC):
                    nc.tensor.matmul(pgu[:, 0:T], lhsT=wg_sb[0][:, c, ic * P:(ic + 1) * P],
                                     rhs=xtb_sb[:, c, :],
                                     start=(c == 0), stop=(c == NHC - 1))

            # ---------- router logits: lo correction -------------------
            for tt in range(NTT):
                tsl = slice(tt * P, (tt + 1) * P)
                for c in range(NHC):
                    nc.tensor.matmul(pls[tt][:, 0:E], lhsT=xtlo_sb[:, c, tsl],
                                     rhs=gcat_sb[:, c, 0:E],
                                     start=False, stop=(c == NHC - 1))

            # ---------- routing DVE chain (per token tile) -------------
            rwT_sb = consts.tile([E, T], F32)
            selm16_sb = consts.tile([P, NTT, E], BF)
            selm_f32 = []
            for tt in range(NTT):
                pl = pls[tt]
                lhalf = rt.tile([P, E], F32, name='lhalf', tag='lhalf')
                nc.vector.tensor_copy(lhalf, pl[:, E:2 * E])
                lsum = rt.tile([P, E], F32, name='lsum', tag='lsum')
                nc.vector.tensor_add(lsum, pl[:, 0:E], lhalf)
                scores = rt.tile([P, E], F32, name='scores', tag='scores')
                nc.scalar.activation(scores, lsum, Act.Sigmoid)
                s4c = rt.tile([P, E], F32, name='s4c', tag='s4c')
                nc.vector.tensor_add(s4c, scores, biasb_sb)

                # group score: sum of top-2 of each group of 4
                s4c3 = s4c.rearrange("p (g j) -> p g j", j=GSZ)
                v = [s4c3[:, :, j] for j in range(GSZ)]
                m1 = rt.tile([P, N_GROUP], F32, name='m1', tag='m1')
                n1 = rt.tile([P, N_GROUP], F32, name='n1', tag='n1')
                m2 = rt.tile([P, N_GROUP], F32, name='m2', tag='m2')
                n2 = rt.tile([P, N_GROUP], F32, name='n2', tag='n2')
                nc.vector.tensor_tensor(m1, v[0], v[1], op=Alu.max)
                nc.vector.tensor_tensor(n1, v[0], v[1], op=Alu.min)
                nc.vector.tensor_tensor(m2, v[2], v[3], op=Alu.max)
                nc.vector.tensor_tensor(n2, v[2], v[3], op=Alu.min)
                top1 = rt.tile([P, N_GROUP], F32, name='top1', tag='top1')
                mn = rt.tile([P, N_GROUP], F32, name='mn', tag='mn')
                mx2 = rt.tile([P, N_GROUP], F32, name='mx2', tag='mx2')
                sec = rt.tile([P, N_GROUP], F32, name='sec', tag='sec')
                nc.vector.tensor_tensor(top1, m1, m2, op=Alu.max)
                nc.vector.tensor_tensor(mn, m1, m2, op=Alu.min)
                nc.vector.tensor_tensor(mx2, n1, n2, op=Alu.max)
                nc.vector.tensor_tensor(sec, mn, mx2, op=Alu.max)
                gsc = rt.tile([P, N_GROUP], F32, name='gsc', tag='gsc')
                nc.vector.tensor_add(gsc, top1, sec)

                g8 = rt.tile([P, 8], F32, name='g8', tag='g8')
                nc.vector.max(g8, gsc)
                gmask = rt.tile([P, N_GROUP], F32, name='gmask', tag='gmask')
                nc.vector.tensor_scalar(gmask, gsc, g8[:, 3:4], None, op0=Alu.is_ge)

                masked = rt.tile([P, E], F32, name='masked', tag='masked')
                masked3 = masked.rearrange("p (g j) -> p g j", j=GSZ)
                for j in range(GSZ):
                    nc.vector.tensor_tensor(masked3[:, :, j], v[j], gmask,
                                            op=Alu.mult)
                t8 = rt.tile([P, 8], F32, name='t8', tag='t8')
                nc.vector.max(t8, masked)
                selm = rt.tile([P, E], F32, name='selm', tag='selm')
                nc.vector.tensor_scalar(selm, masked, t8[:, 7:8], None,
                                        op0=Alu.is_ge)
                selm_f32.append(selm)
                nc.vector.tensor_copy(selm16_sb[:, tt, :], selm)

                rw_raw = rt.tile([P, E], F32, name='rw_raw', tag='rw_raw')
                nc.vector.tensor_tensor(rw_raw, scores, selm, op=Alu.mult)
                den = rt.tile([P, 1], F32, name='den', tag='den')
                nc.vector.tensor_reduce(den, rw_raw, axis=mybir.AxisListType.X,
                                        op=Alu.add)
                inv = rt.tile([P, 1], F32, name='inv', tag='inv')
                nc.vector.reciprocal(inv, den)
                rw = rt.tile([P, E], F32, name='rw', tag='rw')
                nc.vector.tensor_scalar(rw, rw_raw, inv,
                                        ROUTED_SCALING_FACTOR,
                                        op0=Alu.mult, op1=Alu.mult)
                for i in range(4):
                    nc.vector.transpose(
                        rwT_sb[:, tt * P + 32 * i:tt * P + 32 * (i + 1)],
                        rw[32 * i:32 * (i + 1), :])
            rwT16 = consts.tile([E, T], BF)
            nc.vector.tensor_copy(rwT16, rwT_sb)

            # ---------- dense expert 0: up proj + silu/copy (Act) ------
            sg0_sb = consts.tile([P, NIC, T], F32)
            pu0_sb = consts.tile([P, NIC, T], F32)
            for ic in range(NIC):
                pgu = pgu0[ic]
                for c in range(NHC):
                    nc.tensor.matmul(pgu[:, T:2 * T], lhsT=wu_sb[0][:, c, ic * P:(ic + 1) * P],
                                     rhs=xtb_sb[:, c, :],
                                     start=(c == 0), stop=(c == NHC - 1))
                nc.scalar.activation(sg0_sb[:, ic, :], pgu[:, 0:T], Act.Silu)
                nc.scalar.copy(pu0_sb[:, ic, :], pgu[:, T:2 * T])

            # ---------- ranks: exclusive prefix-sum over tokens --------
            lstrict = consts.tile([P, P], BF)
            nc.vector.tensor_scalar(lstrict, iota_row, iota_col, None,
                                    op0=Alu.is_gt)
            ones128 = consts.tile([P, P], BF)
            nc.vector.memset(ones128, 1.0)
            # rank PSUM lives in a spare out bank (yps[2] is untouched
            # until down_dense, well after the rank reads complete)
            r2 = yps[2]
            nc.tensor.matmul(r2[:, 0:E], lhsT=lstrict,
                             rhs=selm16_sb[:, 0, :], start=True, stop=True)
            nc.tensor.matmul(r2[:, E:2 * E], lhsT=ones128,
                             rhs=selm16_sb[:, 0, :], start=True, stop=False)
            nc.tensor.matmul(r2[:, E:2 * E], lhsT=lstrict,
                             rhs=selm16_sb[:, 1, :], start=False, stop=True)

            # local ranks (rank+1 encoding): u2 = (R2+1)*selm*lselm, then
            # per-local-expert sum over the 8 group columns (one-hot mask)
            rloc1_sb = consts.tile([P, NTT, E_LOC], F32)
            for tt in range(NTT):
                selmM = rt.tile([P, E], F32, name='selmM', tag='selmM')
                nc.vector.tensor_mul(selmM, selm_f32[tt], lselm_sb)
                u2 = rt.tile([P, E], F32, name='u2', tag='u2')
                nc.vector.scalar_tensor_tensor(
                    u2, r2[:, tt * E:(tt + 1) * E], 1.0, selmM,
                    op0=Alu.add, op1=Alu.mult)
                u2v = u2.rearrange("p (k j) -> p j k", j=E_LOC)
                for j in range(1, E_LOC):
                    nc.vector.tensor_reduce(rloc1_sb[:, tt, j:j + 1],
                                            u2v[:, j, :],
                                            axis=mybir.AxisListType.X,
                                            op=Alu.add)

            # R2' = (R2+1)*selm - 1 (global, for the scatter-side Gw)
            r2p_sb = consts.tile([P, NTT, E], F32)
            for tt in range(NTT):
                u = rt.tile([P, E], F32, name='u', tag='u')
                nc.vector.scalar_tensor_tensor(
                    u, r2[:, tt * E:(tt + 1) * E], 1.0, selm_f32[tt],
                    op0=Alu.add, op1=Alu.mult)
                nc.vector.tensor_scalar(r2p_sb[:, tt, :], u, -1.0, None,
                                        op0=Alu.add)

            # gather one-hot G^T[t, slot] = (rank_e(t)+1 == slot_local+1)
            gT_sb = consts.tile([P, NTT, SLOTS], BF)
            for tt in range(NTT):
                for e in range(1, E_LOC):
                    nc.vector.tensor_scalar(
                        gT_sb[:, tt, (e - 1) * C:e * C], iota_row1,
                        rloc1_sb[:, tt, e:e + 1], None, op0=Alu.is_equal)

            # ---------- token gather: xgT[h, slot] ---------------------
            xgT_sb = consts.tile([P, NHC, SLOTS], BF)
            for hc in range(NHC):
                g = pst(f'g{hc}')
                for tt in range(NTT):
                    nc.tensor.matmul(g[:, 0:SLOTS],
                                     lhsT=xnat_sb[:, tt, hc * P:(hc + 1) * P],
                                     rhs=gT_sb[:, tt, :],
                                     start=(tt == 0), stop=(tt == NTT - 1))
                nc.vector.tensor_copy(xgT_sb[:, hc, :], g[:, 0:SLOTS])

            # ---------- dense expert 0: rw fold (Pool engine) ----------
            rwb0p = pst('rwb0')
            nc.tensor.matmul(rwb0p[:, 0:T], lhsT=selbc_sb[:, 0:P],
                             rhs=rwT16, start=True, stop=True)
            rwb0_sb = consts.tile([P, T], F32)
            nc.scalar.copy(rwb0_sb, rwb0p[:, 0:T])
            at0 = atp.tile([P, NIC, T], BF, name='at0', tag='at0')
            for ic in range(NIC):
                t1d = rt.tile([P, T], F32, name=f't1d{ic}', tag='t1d')
                nc.gpsimd.tensor_mul(t1d, sg0_sb[:, ic, :], pu0_sb[:, ic, :])
                nc.gpsimd.tensor_mul(at0[:, ic, :], t1d, rwb0_sb)

            # ---------- transposed ranks (scatter-side, off-chain) -----
            r2T_sb = consts.tile([E, T], F32)
            for tt in range(NTT):
                for i in range(4):
                    nc.vector.transpose(
                        r2T_sb[:, tt * P + 32 * i:tt * P + 32 * (i + 1)],
                        r2p_sb[32 * i:32 * (i + 1), tt, :])
            r2T16 = consts.tile([E, T], BF)
            nc.vector.tensor_copy(r2T16, r2T_sb)

            # ---------- sparse experts ---------------------------------
            at_sb = {}
            ygsb = {}
            pgs = {}

            def emit_gu(e):
                pg = pst(f'pg{e}')
                pu = pst(f'pu{e}')
                pgs[e] = (pg, pu)
                xg = xgT_sb[:, :, (e - 1) * C:e * C]
                for it in range(NIC):
                    for hc in range(NHC):
                        nc.tensor.matmul(pg[:, it * C:(it + 1) * C],
                                         lhsT=wg_sb[e][:, hc, it * P:(it + 1) * P],
                                         rhs=xg[:, hc, :],
                                         start=(hc == 0), stop=(hc == NHC - 1))
                for it in range(NIC):
                    for hc in range(NHC):
                        nc.tensor.matmul(pu[:, it * C:(it + 1) * C],
                                         lhsT=wu_sb[e][:, hc, it * P:(it + 1) * P],
                                         rhs=xg[:, hc, :],
                                         start=(hc == 0), stop=(hc == NHC - 1))

            def emit_at(e):
                pg, pu = pgs[e]
                sg = actp.tile([P, NIC * C], F32, name=f'sg{e}', tag='sg')
                nc.scalar.activation(sg, pg, Act.Silu)
                at = atp.tile([P, NIC, C], BF, name=f'at{e}', tag='at', bufs=2)
                nc.vector.tensor_mul(at.rearrange("p i c -> p (i c)"), sg, pu)
                at_sb[e] = at

            gw_sb = consts.tile([P, N_SP, T], BF)

            def emit_gw(e):
                rbc = pst(f'rbc{e}')
                nc.tensor.matmul(rbc[:, 0:T],
                                 lhsT=selbc_sb[:, e * P:(e + 1) * P],
                                 rhs=r2T16, start=True, stop=True)
                nc.tensor.matmul(rbc[:, T:2 * T],
                                 lhsT=selbc_sb[:, e * P:(e + 1) * P],
                                 rhs=rwT16, start=True, stop=True)
                eq = rt.tile([P, T], F32, name=f'eq{e}', tag='eq')
                nc.vector.tensor_scalar(eq, rbc[:, 0:T], iota_col, None,
                                        op0=Alu.is_equal)
                nc.vector.tensor_tensor(gw_sb[:, e - 1, :], eq, rbc[:, T:2 * T],
                                        op=Alu.mult)

            def emit_down_dense():
                # expert 0 dense: writes [t, h] directly into out PSUM
                for tt in range(NTT):
                    for hh in range(HH):
                        for it in range(NIC):
                            nc.tensor.matmul(
                                yps[tt * HH + hh],
                                lhsT=at0[:, it, tt * P:(tt + 1) * P],
                                rhs=wd_sb[0][:, it, hh * 512:(hh + 1) * 512],
                                start=(it == 0), stop=False)

            def emit_down(e, hh_list=None):
                if e not in ygsb:
                    ygsb[e] = ygp.tile([P, HH, 512], BF, name=f'ygsb{e}',
                                       tag='ygsb', bufs=2)
                yg = ygsb[e]
                for hh in (hh_list if hh_list is not None else range(HH)):
                    p = pst(f'yg{e}_{hh}')
                    for it in range(NIC):
                        nc.tensor.matmul(p, lhsT=at_sb[e][:, it, :],
                                         rhs=wd_sb[e][:, it, hh * 512:(hh + 1) * 512],
                                         start=(it == 0), stop=(it == NIC - 1))
                    if hh == 0:
                        nc.vector.tensor_copy(yg[:, hh, :], p)
                    else:
                        nc.scalar.copy(yg[:, hh, :], p)

            osbs = [outp.tile([P, 512], BF, name=f'osb{tt}_{hh}',
                              tag=f'osb{tt}_{hh}')
                    for tt in range(NTT) for hh in range(HH)]

            def drain_out(tt, hh):
                k = tt * HH + hh
                if (tt + hh) % 2 == 0:
                    nc.vector.tensor_copy(osbs[k], yps[k])
                else:
                    nc.scalar.copy(osbs[k], yps[k])
                nc.sync.dma_start(
                    out_d[tt * P:(tt + 1) * P, hh * 512:(hh + 1) * 512],
                    osbs[k])

            def emit_scatter(e, hh_list=None, drain=False):
                last = (e == E_LOC - 1)
                for hh in (hh_list if hh_list is not None else range(HH)):
                    for tt in range(NTT):
                        nc.tensor.matmul(
                            yps[tt * HH + hh],
                            lhsT=gw_sb[:, e - 1, tt * P:(tt + 1) * P],
                            rhs=ygsb[e][:, hh, :],
                            start=False, stop=last)
                    if drain:
                        for tt in range(NTT):
                            drain_out(tt, hh)

            # PE order: dense expert 0 first (gu emitted above), sparse
            # experts pipelined behind their weight DMAs; wd3 split by
            # h-half so the tail after the last DMA byte is minimal.
            emit_gu(1)
            emit_down_dense()
            for e in range(1, E_LOC):
                emit_gw(e)
            emit_at(1)
            emit_gu(2)
            emit_at(2)
            emit_down(1)
            emit_scatter(1)
            emit_gu(3)
            emit_at(3)
            emit_down(2)
            emit_scatter(2)
            emit_down(3, [0])
            emit_scatter(3, [0], drain=True)
            emit_down(3, [1])
            emit_scatter(3, [1], drain=True)

    _spill_excess_waits(nc)
    return nc


def _spill_excess_waits(nc, max_waits=1):
    """walrus codegen in this container accepts at most one semaphore wait
    per engine instruction; move extra waits onto preceding same-engine NOPs
    (engine queues are in-order, so this preserves the synchronization)."""
    f = nc.m.functions[0]
    for b in f.blocks:
        new_insts = []
        for inst in b.instructions:
            si = inst.sync_info
            if si is not None and si.on_wait is not None \
                    and len(si.on_wait) > max_waits:
                waits = list(si.on_wait)
                keep = waits[-max_waits:]
                extra = waits[:-max_waits]
                for k, w in enumerate(extra):
                    nop = mybir.InstNoOp(
                        name=f"{inst.name}-wspill{k}",
                        sync_info=mybir.SyncInfo(on_wait=[w], on_update=[]),
                        bass_nofuse=True,
                        engine=inst.engine,
                    )
                    new_insts.append(nop)
                inst.sync_info = mybir.SyncInfo(
                    on_wait=keep, on_update=list(si.on_update or []))
            new_insts.append(inst)
        b.instructions = new_insts


def kernel(x, gate_w, e_score_bias, Wg, Wu, Wd):
    if 'nc' not in _CACHE:
        _CACHE['nc'] = _build()
    nc = _CACHE['nc']

    f16 = np.float16

    def pmajor_ht(a):
        n = a.shape[1]
        return np.ascontiguousarray(
            a.reshape(NHC, P, n).transpose(1, 0, 2).reshape(P, NHC * n))

    xf = np.asarray(x).astype(np.float32)
    xT = np.ascontiguousarray(xf.T)
    xTb = xT.astype(f16)
    xTlo = (xT - xTb.astype(np.float32)).astype(f16)
    xnat = np.ascontiguousarray(
        xf.astype(f16).reshape(NTT, P, H).transpose(1, 0, 2).reshape(P, -1))
    gate = np.ascontiguousarray(np.asarray(gate_w)).astype(np.float32)
    ghi = gate.astype(f16)
    glo = (gate - ghi.astype(np.float32)).astype(f16)
    gcat = np.concatenate([ghi, glo], axis=1)          # [H, 2E]
    biasb = np.broadcast_to(
        np.asarray(e_score_bias).astype(np.float32)[None, :], (P, E)).copy()
    Wgb = np.asarray(Wg).astype(f16).reshape(E, NHC, P, I)
    Wgb = np.ascontiguousarray(Wgb.transpose(2, 0, 1, 3))      # [P,E,NHC,I]
    Wub = np.asarray(Wu).astype(f16).reshape(E, NHC, P, I)
    Wub = np.ascontiguousarray(Wub.transpose(2, 0, 1, 3))
    Wdb = np.asarray(Wd).astype(f16).reshape(E, NIC, P, H)
    Wdb = np.ascontiguousarray(Wdb.transpose(2, 0, 1, 3))      # [P,E,NIC,H]

    in_maps = []
    for c in range(N_CORES):
        sel = np.zeros((E, E_LOC, P), dtype=f16)
        lselm = np.zeros((E,), dtype=np.float32)
        for j in range(E_LOC):
            sel[c * E_LOC + j, j, :] = 1.0
            lselm[c * E_LOC + j] = 1.0
        esl = slice(c * E_LOC, (c + 1) * E_LOC)
        in_maps.append({
            'xtb': pmajor_ht(xTb),
            'xtlo': pmajor_ht(xTlo),
            'gcat': pmajor_ht(gcat),
            'biasb': biasb,
            'xnat': xnat,
            'selbc': sel.reshape(E, E_LOC * P),
            'lselm': np.broadcast_to(lselm[None, :], (P, E)).copy(),
            'wg': np.ascontiguousarray(Wgb[:, esl]).reshape(P, -1),
            'wu': np.ascontiguousarray(Wub[:, esl]).reshape(P, -1),
            'wd': np.ascontiguousarray(Wdb[:, esl]).reshape(P, -1),
        })

    _CACHE['in_maps'] = in_maps
    res = run_bass_kernel_spmd(nc, in_maps, core_ids=list(range(N_CORES)))
    out = np.zeros((T, H), dtype=np.float32)
    for c in range(N_CORES):
        out += res.results[c]['out'].astype(np.float32)
    return out


def run_traced(**kwargs):
    """Re-run the last kernel invocation with NTFF tracing enabled."""
    return run_bass_kernel_spmd(_CACHE['nc'], _CACHE['in_maps'],
                                core_ids=list(range(N_CORES)), trace=True,
                                **kwargs)
